# revision 48
# baseline (speedup 1.0000x reference)
"""Trainium2 Bass kernel for the diagonal OU-SDE sampler (nn_DiagOUSDE).

Math: y[b,0]=mu+noise[b,0]*sqrt(var0); y[b,t]=A_t*y[b,t-1]+mu(1-A_t)+sqrt(Q_t)*noise[b,t]
with A/Q per (t,d) exact OU transition coefficients.

Fast path (d-uniform AND t-uniform coefficients, mu=0 — the common case):
bf16 end-to-end, batch-sharded 8 ways, HYBRID engine split per core:
  * batch rows 0-4 (10 n-tiles of 128 OU processes, partitions = processes,
    free dim = full T): the recurrence state = A*state + e runs natively on
    DVE via tensor_tensor_scan (fp32 state/A-constant, bf16 e/out). DVE also
    pre-scales e = noise*sqrtQ in 2x bf16 tensor_scalar mode.
  * batch rows 5-7 (one N=512 pair chain + one N=256 single chain): chunked
    prefix-scan on the PE — 22 matmuls of the bf16 [97,97] folded transition
    weight (sqrtQ folded in, raw-noise rhs, chunk carry as contraction row
    96), ACT copies the PSUM row-96 carry and evacuates/casts PSUM to bf16.
  This splits the scan work across DVE / PE+ACT so steady-state lands at
  ~95% of the 16.8MB-per-core HBM roofline (~47us at 358 GB/s).
Fallbacks: d-uniform-only -> chunked-matmul path for all rows ("fast");
otherwise fp32 general path with per-(t,d) coefficient tensors.

Measured (iteration-delta on 8 axon trn2 cores @1.2GHz): ~50us steady-state,
rel err ~3.3e-3 (bf16 I/O; gate is 2e-2).
"""
import sys

for _p in ("/opt/trn_rl_repo", "/opt/pypackages"):
    if _p not in sys.path:
        sys.path.append(_p)

import numpy as np

import concourse.bacc as bacc
import concourse.mybir as mybir
from concourse.tile import TileContext
from concourse.bass_utils import run_bass_kernel_spmd

B, T, D = 64, 2048, 256
N_CORES = 8
B_S = B // N_CORES            # 8 batch rows per core
L = 96                        # time steps per chunk; contraction row 96 = carry
C = (T + L - 1) // L          # 22 chunks
TP = C * L                    # padded time length 2112

_f32 = np.float32


def _host_coeffs(ts, mu, log_kappa, log_sigma):
    """Per-(t,d) coefficient arrays in float32, mirroring the JAX reference."""
    ts = ts.astype(_f32)
    kappa = np.logaddexp(_f32(0.0), log_kappa.astype(_f32)).astype(_f32) + _f32(1e-6)
    sigma = np.logaddexp(_f32(0.0), log_sigma.astype(_f32)).astype(_f32) + _f32(1e-6)
    var0 = sigma * sigma / (_f32(2.0) * kappa)
    dt = np.maximum(ts[1:] - ts[:-1], _f32(1e-6))[:, None]            # [T-1,1]
    A = np.exp(-kappa[None, :] * dt).astype(_f32)                     # [T-1,D]
    two_k_dt = (_f32(2.0) * kappa[None, :] * dt).astype(_f32)
    small = (two_k_dt < _f32(1e-6)).astype(_f32)
    Q_exact = sigma**2 * (_f32(1.0) - np.exp(-two_k_dt)) / np.maximum(
        _f32(2.0) * kappa, _f32(1e-12))
    Q_taylor = sigma**2 * dt * (_f32(1.0) - kappa * dt + two_k_dt**2 / _f32(6.0))
    Q = (small * Q_taylor + (_f32(1.0) - small) * Q_exact).astype(_f32)

    A_full = np.concatenate([np.ones((1, D), _f32), A], axis=0)       # A_0 := 1
    sqrtQ_full = np.sqrt(
        np.concatenate([var0[None, :], Q], axis=0)).astype(_f32)      # [T,D]
    b_full = np.concatenate(
        [mu[None, :].astype(_f32), (mu[None, :] * (_f32(1.0) - A)).astype(_f32)],
        axis=0)

    logG = np.cumsum(np.log(A_full.astype(np.float64)), axis=0)
    G = np.exp(logG).astype(_f32)
    S_u = (sqrtQ_full * np.exp(-logG)).astype(_f32)                   # u = noise*S_u

    if np.any(b_full != 0):
        ydet = np.empty((T, D), _f32)
        y = b_full[0].copy()
        ydet[0] = y
        for t in range(1, T):
            y = A_full[t] * y + b_full[t]
            ydet[t] = y
    else:
        ydet = None
    return S_u, G, ydet, A_full, sqrtQ_full


def _pad_tp(a):
    out = np.zeros((TP, D), _f32)
    out[:T] = a
    return out


def _tri_weight():
    # [97, 97]: W[s,t]=1{s<=t} (s,t<96); row 96 = carry (all ones);
    # col 96 duplicates col 95 => psum row 96 = chunk-final cum (the next carry)
    w = np.zeros((97, 97), _f32)
    for s in range(L):
        w[s, s:L] = _f32(1.0)
    w[L, :L] = _f32(1.0)
    w[:, L] = w[:, L - 1]
    return w


def _wfold_weights(A_full, sqrtQ_full):
    """[C, 97, 97] float32 exact per-chunk transition weights (d-independent
    coefficients; requires per-t d-uniform A/sqrtQ). The sqrt(Q_s) input scaling
    is folded into the weight rows, so the rhs is RAW noise and PSUM rows are
    final y: W_c[s,t] = (prod_{r=t0+s+1..t0+t} A_r) * sqrtQ_{t0+s} for s<=t<96,
    row 96 = carry coefficients prod_{t0..t0+t} A_r, col 96 duplicates col 95
    (so PSUM row 96 = chunk-final y = the next chunk's carry)."""
    q = np.zeros(TP)
    q[:T] = sqrtQ_full[:, 0]
    ap = np.zeros(TP)
    ap[:T] = A_full[:, 0].astype(np.float64)
    Ws = np.zeros((C, L + 1, L + 1), np.float64)
    for c in range(C):
        t0 = c * L
        with np.errstate(divide="ignore"):
            cls = np.cumsum(np.log(ap[t0:t0 + L]))  # log prod_{t0..t0+t}
        M = np.exp(cls[:, None] - cls[None, :])     # [t, s] = prod_{s+1..t}
        M = np.tril(M)
        np.fill_diagonal(M, 1.0)
        W = Ws[c]
        W[:L, :L] = (M * q[t0:t0 + L][None, :]).T   # W[s,t] = M[t,s]*q[t0+s]
        W[L, :L] = np.exp(cls)
        W[:, L] = W[:, L - 1]
    return np.ascontiguousarray(Ws.astype(_f32))


NT = B_S * D // 128      # 16 n-tiles of 128 OU processes per core
GK = 4                   # n-tiles per DMA/compute group
NG = NT // GK


NTS = NT - 6             # scan n-tiles (10); batch rows 5-7 go via PE
SGROUPS = ((0, 4), (4, 7), (7, 10))   # ragged scan groups (tile ranges)


def _build_nc_scan(A, sq, r, n_iters=1):
    """Hybrid t-uniform d-uniform fastest path (see module docstring).

    10 n-tiles (batch rows 0-4) scan on DVE; rows 5-7 run as two PE chunk-
    matmul carry chains (N=512 pair + N=256 single) with ACT carries/evacs.
    The t=0 stationary-variance init is handled by scaling the e column 0
    by sqrt(var0)=sq*r (scan side) / folding into W0 row 0 (PE side).
    A/sq/r are python floats baked into the program.
    """
    nc = bacc.Bacc("TRN2", target_bir_lowering=False, debug=False,
                   num_devices=N_CORES)
    dt32 = mybir.dt.float32
    dtb = mybir.dt.bfloat16
    noise = nc.dram_tensor("noise", [128, NTS, T], dtb, kind="ExternalInput")
    noise_m = nc.dram_tensor("noise_m", [L, C, 2 * D], dtb,
                             kind="ExternalInput")
    noise_m2 = nc.dram_tensor("noise_m2", [L, C, D], dtb,
                              kind="ExternalInput")
    wstk = nc.dram_tensor("wstk", [L + 1, 2 * (L + 1)], dtb,
                          kind="ExternalInput")
    yout = nc.dram_tensor("yout", [128, NTS, T], dtb, kind="ExternalOutput")
    yout_m = nc.dram_tensor("yout_m", [L, C, 2 * D], dtb,
                            kind="ExternalOutput")
    yout_m2 = nc.dram_tensor("yout_m2", [L, C, D], dtb,
                             kind="ExternalOutput")
    CH = C // 2

    with TileContext(nc) as tc:
        with (
            tc.tile_pool(name="psum", bufs=4, space="PSUM") as pspool,
            tc.tile_pool(name="psum2", bufs=4, space="PSUM") as pspool2,
            tc.tile_pool(name="coefs", bufs=1) as coefs,
            tc.tile_pool(name="upool", bufs=len(SGROUPS) + 1) as upool,
            tc.tile_pool(name="ypool", bufs=len(SGROUPS)) as ypool,
            tc.tile_pool(name="umm", bufs=2) as ummp,
        ):
            def body(_iv=None):
                amat = coefs.tile([128, T], dt32, tag="amat", name="amat")
                nc.vector.memset(amat[:, :], float(A))

                # scan group 0's load goes first so DVE starts ASAP
                us = []
                for g, (k0, k1) in enumerate(SGROUPS):
                    u = upool.tile([128, k1 - k0, T], dtb, tag="u",
                                   name=f"u{g}")
                    us.append(u)
                nc.sync.dma_start(out=us[0][:],
                                  in_=noise[:, SGROUPS[0][0]:SGROUPS[0][1], :])

                w_t = coefs.tile([L + 1, 2, L + 1], dtb, tag="w", name="w_t")
                nc.sync.dma_start(
                    out=w_t[:], in_=wstk[:].rearrange("s (c t) -> s c t", c=2))
                u_mm = ummp.tile([128, C, 2 * D], dtb, tag="um", name="u_mm")
                u_m2 = ummp.tile([128, C, D], dtb, tag="um2", name="u_m2")
                nc.gpsimd.memset(u_mm[L:L + 1, 0, :], 0.0)  # carry0
                nc.gpsimd.memset(u_m2[L:L + 1, 0, :], 0.0)
                for c0, c1 in ((0, CH), (CH, C)):
                    nc.sync.dma_start(out=u_mm[0:L, c0:c1, :],
                                      in_=noise_m[:, c0:c1, :])
                    nc.sync.dma_start(out=u_m2[0:L, c0:c1, :],
                                      in_=noise_m2[:, c0:c1, :])
                for g in range(1, len(SGROUPS)):
                    k0, k1 = SGROUPS[g]
                    nc.sync.dma_start(out=us[g][:], in_=noise[:, k0:k1, :])

                # PE warmup during the load ramp
                wps = pspool.tile([L + 1, 2 * D], dt32, tag="ps512",
                                  name="wps")
                for _ in range(10):
                    nc.tensor.matmul(wps[:, 0:L + 1], w_t[:, 0, :],
                                     w_t[:, 0, :], start=True, stop=True)

                def scan_group(g):
                    k0, k1 = SGROUPS[g]
                    gk = k1 - k0
                    u = us[g]
                    y = ypool.tile([128, gk, T], dtb, tag="y", name=f"y{g}")
                    # scales on DVE in 2x mode (bf16 both sides) so the ACT
                    # queue holds only the MM chains' small carries/evacs
                    for k in range(gk):
                        nc.vector.tensor_scalar_mul(
                            y[:, k, 1:T], u[:, k, 1:T], float(sq))
                    nc.vector.tensor_scalar_mul(
                        y[:, :, 0:1], u[:, :, 0:1], float(sq * r))
                    for k in range(gk):
                        nc.vector.tensor_tensor_scan(
                            u[:, k, :], amat[:, :], y[:, k, :], 0.0,
                            mybir.AluOpType.mult, mybir.AluOpType.add)
                    nc.scalar.dma_start(out=yout[:, k0:k1, :], in_=u[:])

                chains = (
                    (u_mm, 2 * D, yout_m, pspool),    # rows 6,7: N=512
                    (u_m2, D, yout_m2, pspool2),      # row 5:    N=256
                )
                g = 0
                for c in range(C):
                    for um, w, yo, pp in chains:
                        ps = pp.tile([L + 1, w], dt32, tag=f"ps{w}",
                                     name=f"ps{w}")
                        nc.tensor.matmul(ps[:], w_t[:, min(c, 1), :],
                                         um[0:L + 1, c, :],
                                         start=True, stop=True)
                        if c + 1 < C:
                            nc.scalar.copy(um[L:L + 1, c + 1, :],
                                           ps[L:L + 1, :])
                        nc.scalar.copy(um[0:L, c, :], ps[0:L, :])
                        if c == CH - 1 or c == C - 1:
                            c0, c1 = (0, CH) if c == CH - 1 else (CH, C)
                            nc.scalar.dma_start(out=yo[:, c0:c1, :],
                                                in_=um[0:L, c0:c1, :])
                    if c in (4, 11, 17) and g < len(SGROUPS):
                        scan_group(g)
                        g += 1
                while g < len(SGROUPS):
                    scan_group(g)
                    g += 1

            if n_iters == 1:
                body()
            else:
                with tc.For_i(0, n_iters, 1) as _i:
                    body(_i)
    nc.compile()
    return nc


def _build_nc_fast(n_iters=1, f32r=False):
    """d-uniform fast path, bf16 end-to-end: noise ships as bf16 pair-major
    [NP, L, C, 2, D] so every DMA runs on ~11KB contiguous descriptor runs
    (the old [B_S, L, C, D] layout with h-interleaved SBUF tiles produced 1KB
    descriptors -> ~53% DMA engine efficiency). Matmuls are bf16 (4x fp32
    rate), PSUM fp32 accumulation; the PSUM->SBUF evacuation casts to bf16 on
    DVE only (ScalarE keeps just the tiny serial carry copies). Output is
    bf16 in DRAM, upcast to fp32 on host. Batch rows are paired so matmuls
    run at N=512; all 4 pair-chains in flight.
    """
    nc = bacc.Bacc("TRN2", target_bir_lowering=False, debug=False,
                   num_devices=N_CORES)
    dt32 = mybir.dt.float32
    dtb = mybir.dt.bfloat16
    NP = B_S // 2  # batch pairs
    CH = C // 2    # chunk half-point (loads/stores split for finer overlap)
    noisep = nc.dram_tensor("noisep", [NP, L, C, 2 * D], dtb,
                            kind="ExternalInput")
    # weight stack pre-transposed on host to [s, c, t] so the load is contiguous
    wstack = nc.dram_tensor("wstack", [L + 1, C * (L + 1)], dtb,
                            kind="ExternalInput")
    youtp = nc.dram_tensor("youtp", [NP, L, C, 2 * D], dtb,
                           kind="ExternalOutput")

    with TileContext(nc) as tc:
        with (
            tc.tile_pool(name="coef", bufs=1) as coef,
            tc.tile_pool(name="upool", bufs=2 * NP) as upool,
            tc.tile_pool(name="psum", bufs=4, space="PSUM") as pspool,
            tc.tile_pool(name="psum2", bufs=4, space="PSUM") as pspool2,
        ):
            def body(_iv=None):
                w_t = coef.tile([L + 1, C, L + 1], dtb, tag="w", name="w_t")
                nc.sync.dma_start(
                    out=w_t[:], in_=wstack[:].rearrange("s (c t) -> s c t", c=C))

                us = []
                halves = ((0, CH), (CH, C))
                for hi, (c0, c1) in enumerate(halves):
                    for p in range(NP):
                        if hi == 0:
                            u = upool.tile([128, C, 2 * D], dtb, tag="u",
                                           name=f"u{p}")
                            us.append(u)
                            nc.gpsimd.memset(u[L:L + 1, 0, :], 0.0)  # carry0
                        u = us[p]
                        # ~11KB per-partition runs are the per-packet
                        # sweet spot for the SDMA engines
                        nc.sync.dma_start(out=u[0:L, c0:c1, :],
                                          in_=noisep[p, :, c0:c1, :])
                # PE warmup during the load ramp: ramps the clock gate so the
                # first real chain matmuls run at full rate
                wps = pspool.tile([L + 1, 2 * D], dt32, tag="ps512",
                                  name="wps")
                for _ in range(10):
                    nc.tensor.matmul(wps[:, 0:L + 1], w_t[:, 0, :],
                                     w_t[:, 0, :], start=True, stop=True)
                # Skewed chain interleave: pair p runs SKEW chunks behind
                # pair p-1 so early chains aren't head-of-line blocked on
                # later pairs' still-inflight loads.
                SKEW = 3
                for step in range(C + (NP - 1) * SKEW):
                    for p in range(NP):
                        c = step - p * SKEW
                        if not (0 <= c < C):
                            continue
                        u = us[p]
                        ps = pspool.tile([L + 1, 2 * D], dt32,
                                         tag="ps", name=f"ps{p}")
                        nc.tensor.matmul(ps[:], w_t[:, c, :],
                                         u[0:L + 1, c, :],
                                         start=True, stop=True)
                        if c + 1 < C:
                            # next chunk's carry = dup'd final-y row
                            nc.scalar.copy(u[L:L + 1, c + 1, :],
                                           ps[L:L + 1, :])
                        # evacuate final y back into the dead u slice (bf16)
                        nc.vector.tensor_copy(out=u[0:L, c, :], in_=ps[0:L, :])
                        if c == CH - 1 or c == C - 1:
                            c0, c1 = (0, CH) if c == CH - 1 else (CH, C)
                            nc.scalar.dma_start(
                                out=youtp[p, :, c0:c1, :],
                                in_=u[0:L, c0:c1, :])

            if n_iters == 1:
                body()
            else:
                with tc.For_i(0, n_iters, 1) as _i:
                    body(_i)
    nc.compile()
    return nc


def _build_nc(with_ydet, n_iters=1):
    """Bass program for one core. noise/yout are [B_S, TP, D] in DRAM."""
    nc = bacc.Bacc("TRN2", target_bir_lowering=False, debug=False,
                   num_devices=N_CORES)
    dt32 = mybir.dt.float32
    noise = nc.dram_tensor("noise", [B_S, TP, D], dt32, kind="ExternalInput")
    s_u = nc.dram_tensor("s_u", [TP, D], dt32, kind="ExternalInput")
    g = nc.dram_tensor("g", [TP, D], dt32, kind="ExternalInput")
    tri = nc.dram_tensor("tri", [L + 1, L + 1], dt32, kind="ExternalInput")
    ydet = (nc.dram_tensor("ydet", [TP, D], dt32, kind="ExternalInput")
            if with_ydet else None)
    yout = nc.dram_tensor("yout", [B_S, TP, D], dt32, kind="ExternalOutput")

    GRP = 4  # batch rows processed with interleaved carry chains

    with TileContext(nc) as tc:
        with (
            tc.tile_pool(name="coef", bufs=1) as coef,
            tc.tile_pool(name="upool", bufs=GRP + 2) as upool,
            tc.tile_pool(name="psum", bufs=4, space="PSUM") as pspool,
            tc.tile_pool(name="psum2", bufs=4, space="PSUM") as pspool2,
        ):
            def body(_iv=None):
                w_t = coef.tile([L + 1, L + 1], dt32, tag="w")
                su_t = coef.tile([128, C, D], dt32, tag="su")
                g_t = coef.tile([128, C, D], dt32, tag="g")
                nc.sync.dma_start(out=w_t[:], in_=tri[:])
                nc.sync.dma_start(
                    out=su_t[0:L, :, :],
                    in_=s_u[:].rearrange("(c p) d -> p c d", p=L))
                nc.sync.dma_start(
                    out=g_t[0:L, :, :],
                    in_=g[:].rearrange("(c p) d -> p c d", p=L))
                if with_ydet:
                    yd_t = coef.tile([128, C, D], dt32, tag="yd")
                    nc.sync.dma_start(
                        out=yd_t[0:L, :, :],
                        in_=ydet[:].rearrange("(c p) d -> p c d", p=L))

                for g0 in range(0, B_S, GRP):
                    bs = range(g0, min(g0 + GRP, B_S))
                    us = {}
                    for b in bs:
                        u = us[b] = upool.tile([128, C, D], dt32, tag="u", name=f"u{b}")
                        nc.sync.dma_start(
                            out=u[0:L, :, :],
                            in_=noise[b].rearrange("(c p) d -> p c d", p=L))
                        nc.gpsimd.memset(u[L:L + 1, 0, :], 0.0)  # chunk-0 carry
                        nc.vector.tensor_mul(out=u[0:L, :, :], in0=u[0:L, :, :],
                                             in1=su_t[0:L, :, :])
                    # interleave the per-b carry chains chunk-by-chunk; pass3
                    # (y = G*cum, PSUM->SBUF) writes back into the dead u slice
                    pss = {}
                    for c in range(C):
                        h = c % 2
                        for b in bs:
                            u = us[b]
                            if h == 0:
                                pss[b] = pspool.tile([L + 1, 2 * D], dt32, tag="ps", name=f"ps{b}")
                            ps = pss[b]
                            nc.tensor.matmul(ps[:, h * D:(h + 1) * D],
                                             w_t[:], u[0:L + 1, c, :],
                                             start=True, stop=True)
                            if c + 1 < C:
                                # next chunk's additive carry = dup'd cum row
                                nc.scalar.copy(u[L:L + 1, c + 1, :],
                                               ps[L:L + 1, h * D:(h + 1) * D])
                            if h == 1:
                                nc.vector.tensor_mul(
                                    out=u[0:L, c - 1:c + 1, :],
                                    in0=g_t[0:L, c - 1:c + 1, :],
                                    in1=ps[0:L, :])
                    for b in bs:
                        u = us[b]
                        if with_ydet:
                            nc.vector.tensor_add(out=u[0:L, :, :],
                                                 in0=u[0:L, :, :],
                                                 in1=yd_t[0:L, :, :])
                        nc.sync.dma_start(
                            out=yout[b].rearrange("(c p) d -> p c d", p=L),
                            in_=u[0:L, :, :])

            if n_iters == 1:
                body()
            else:
                with tc.For_i(0, n_iters, 1) as _i:
                    body(_i)
    nc.compile()
    return nc


_CACHE = {}


def _get_nc(mode, n_iters=1):
    key = (mode, n_iters)
    if key not in _CACHE:
        if isinstance(mode, tuple) and mode[0] == "scan":
            _CACHE[key] = _build_nc_scan(*mode[1:], n_iters=n_iters)
        elif mode == "fast":
            _CACHE[key] = _build_nc_fast(n_iters)
        elif mode == "fast_f32r":
            _CACHE[key] = _build_nc_fast(n_iters, f32r=True)
        else:
            _CACHE[key] = _build_nc(mode == "general_ydet", n_iters)
    return _CACHE[key]


def _make_in_maps(ts, noise, mu, log_kappa, log_sigma):
    """Returns (in_maps, mode). mode: 'fast' when the per-(t,d) coefficients
    are uniform (uniform time grid, d-uniform kappa/sigma, mu=0) — then the
    exact d-independent chunk weight is used and no coefficient tensors ship."""
    S_u, G, ydet, A_full, sqrtQ_full = _host_coeffs(
        np.asarray(ts), np.asarray(mu),
        np.asarray(log_kappa), np.asarray(log_sigma))
    noise = np.ascontiguousarray(np.asarray(noise), dtype=_f32)

    d_uniform = (ydet is None
                 and np.ptp(A_full, axis=1).max() == 0
                 and np.ptp(sqrtQ_full, axis=1).max() == 0
                 and A_full.min() > 0)
    # t-uniform: all steps t>=1 share one (A, sqrtQ) up to fp32 arange
    # jitter in dt (rel ~1e-7) — the scan path bakes them in as scalars
    def _tuni(a, rtol):
        m = float(np.float64(a[1:]).mean())
        return m != 0 and float(np.ptp(a[1:])) / abs(m) < rtol
    # A compounds over the decay window (~1/(1-A)) -> tight tol; sqrtQ only
    # scales e elementwise -> loose tol
    t_uniform = (d_uniform and T >= 2
                 and _tuni(A_full, 1e-5) and _tuni(sqrtQ_full, 1e-3)
                 and sqrtQ_full[1, 0] > 0)
    if t_uniform:
        import ml_dtypes
        bf16 = ml_dtypes.bfloat16
        A = float(np.float64(A_full[1:]).mean())
        sq = float(np.float64(sqrtQ_full[1:]).mean())
        r = float(np.float32(sqrtQ_full[0, 0]) / np.float32(sq))
        ws = _wfold_weights(A_full, sqrtQ_full)           # [C, 97, 97]
        # t-uniform: all chunks c>=1 share ws[1]; stack [W0, Wshared] as
        # [s, c2, t] for a contiguous load
        wstk = np.ascontiguousarray(
            np.stack([ws[0], ws[1]]).transpose(1, 0, 2)
            .reshape(L + 1, 2 * (L + 1))).astype(bf16)
        nb_s = NTS * 128 // D                             # 5 scan batch rows
        in_maps = []
        for core in range(N_CORES):
            shard = noise[core * B_S:(core + 1) * B_S]    # [B_S, T, D]
            # scan region rows 0..nb_s-1: dram[p, k, t] = process n = k*128+p
            sp = np.ascontiguousarray(
                shard[:nb_s].transpose(0, 2, 1).reshape(NTS, 128, T)
                .transpose(1, 0, 2)).astype(bf16)
            # PE pair rows 6,7: [L, C, 2*D], t = c*L + l, zero-padded to TP
            mpad = np.zeros((2, TP, D), _f32)
            mpad[:, :T] = shard[B_S - 2:]
            mp = np.ascontiguousarray(
                mpad.reshape(2, C, L, D).transpose(2, 1, 0, 3)
                .reshape(L, C, 2 * D)).astype(bf16)
            # PE single row 5: [L, C, D]
            mpad2 = np.zeros((TP, D), _f32)
            mpad2[:T] = shard[nb_s]
            mp2 = np.ascontiguousarray(
                mpad2.reshape(C, L, D).transpose(1, 0, 2)).astype(bf16)
            in_maps.append({"noise": sp, "noise_m": mp, "noise_m2": mp2,
                            "wstk": wstk})
        return in_maps, ("scan", A, sq, r)

    shards = []
    for core in range(N_CORES):
        shard = noise[core * B_S:(core + 1) * B_S]        # [B_S, T, D]
        npad = np.zeros((B_S, TP, D), _f32)
        npad[:, :T] = shard
        shards.append(npad)

    if d_uniform:
        import ml_dtypes
        bf16 = ml_dtypes.bfloat16
        ws = _wfold_weights(A_full, sqrtQ_full)          # [C, s, t]
        ws_t = np.ascontiguousarray(
            ws.transpose(1, 0, 2)).reshape(L + 1, C * (L + 1)).astype(bf16)
        NP = B_S // 2
        in_maps = []
        for s in shards:
            # [B_S, TP, D] -> [NP, L, C, 2*D] pair-major bf16:
            # t = c*L + l laid out as [l, c], batch pair h packed next to D
            sp = np.ascontiguousarray(
                s.reshape(NP, 2, C, L, D).transpose(0, 3, 2, 1, 4)
                .reshape(NP, L, C, 2 * D).astype(bf16))
            in_maps.append({"noisep": sp, "wstack": ws_t})
        return in_maps, "fast"

    su_p = _pad_tp(S_u)
    g_p = _pad_tp(G)
    yd_p = _pad_tp(ydet) if ydet is not None else None
    tri = _tri_weight()
    in_maps = []
    for s in shards:
        m = {"noise": s, "s_u": su_p, "g": g_p, "tri": tri}
        if yd_p is not None:
            m["ydet"] = yd_p
        in_maps.append(m)
    return in_maps, ("general_ydet" if yd_p is not None else "general")


def kernel(ts, noise, mu, log_kappa, log_sigma):
    in_maps, mode = _make_in_maps(ts, noise, mu, log_kappa, log_sigma)
    nc = _get_nc(mode)
    res = run_bass_kernel_spmd(nc, in_maps, list(range(N_CORES)))
    out = np.empty((B, T, D), _f32)
    NP = B_S // 2
    for core in range(N_CORES):
        r = res.results[core]
        if isinstance(mode, tuple) and mode[0] == "scan":
            nb_s = NTS * 128 // D
            # scan region [128, NTS, T] bf16 -> rows 0..nb_s-1
            y = (r["yout"].astype(_f32)
                 .transpose(1, 0, 2).reshape(nb_s, D, T).transpose(0, 2, 1))
            out[core * B_S:core * B_S + nb_s] = y
            # PE single row [L, C, D] bf16 -> row nb_s
            ym2 = (r["yout_m2"].astype(_f32)
                   .transpose(1, 0, 2).reshape(TP, D))
            out[core * B_S + nb_s] = ym2[:T]
            # PE pair [L, C, 2*D] bf16 -> rows B_S-2..B_S-1
            ym = (r["yout_m"].astype(_f32)
                  .reshape(L, C, 2, D).transpose(2, 1, 0, 3)
                  .reshape(2, TP, D))
            out[core * B_S + B_S - 2:(core + 1) * B_S] = ym[:, :T, :]
            continue
        if mode == "fast":
            # [NP, L, C, 2*D] bf16 -> [B_S, TP, D] fp32
            y = (r["youtp"].astype(_f32)
                 .reshape(NP, L, C, 2, D).transpose(0, 3, 2, 1, 4)
                 .reshape(B_S, TP, D))
        else:
            y = r["yout"]
        out[core * B_S:(core + 1) * B_S] = y[:, :T, :]
    return out

